# revision 51
# baseline (speedup 1.0000x reference)
"""HardTripletLoss Trainium2 kernel.

Reference computation (B=256, C=1000, D=300):
  relations[b,c] = ||emb[b*C+c] - att[b*C+c] + 1e-6||_2          [B, C]
  hardest_positive[c] = max_b relations[b,c] * onehot(labels)[b,c]
  mx[c]              = max_b relations[b,c]
  hardest_negative[c] = min_b (relations[b,c] + mx[c]*onehot[b,c])
  loss = sum(relu(hp - hn + 1)) / (count(relu(...) > 1e-16) + 1e-16)

Sharding: data-parallel over B across 8 cores; core m owns the contiguous
32000-row slice [m*32000, (m+1)*32000) of the (B*C, D) tensors. The device
does the heavy part only: per-row squared distances rel_sq[row] =
sum_d (emb-att+eps)^2, shipped back as [128, 250] f32 (128 KB/core,
+0.3% traffic). The host reshapes to [B, C], takes sqrt, and runs the
reference's tiny [B,C] max/min/mask logic in numpy (~ms).

Performance design (HW exec ~128 us vs 610 us baseline):
- fp16 inputs (host pre-cast): halves device bytes to 38.4 MB/core.
  Final loss err ~3e-5 vs 2e-2 tolerance.
- All bulk loads on SWDGE (gpsimd): packets round-robin over all 16 DMA
  engines. HWDGE queues are pinned to engines 64-68 only and bottleneck
  at ~110 GB/s total.
- Rows-contiguous layout: partition p holds rows [p*250, (p+1)*250), so
  a 25-row chunk DMA is [128, 15 KB] -- 128 descriptors, 15 KB packets.
  Packet size sets the per-engine rate: 4.8 KB packets sustain ~200
  GB/s/core, 15 KB packets ~425 GB/s/core. Few descriptors per DMA also
  matters: SWDGE issue lock-steps on an 8-deep completion-sem ring, and
  250-desc DMAs stalled the issue stream 10-14 us at a time.
- Per chunk: in-place DVE subtract, one ACT Square (bias=eps), two f16
  fold-adds (2 elem/cycle) + one f32 3D row-sum reduce on DVE, emitted
  one chunk behind the subtract (engine streams are in-order; without
  the skew DVE idles waiting for ACT inside every chunk).
- ~127.6 us is the measured structural floor: DVE busy (~96 us; the
  reduce has no DVE fast modes, custom-DVE accum is [P,1]-only) ~=
  DMA stream (~92.5 us; per-engine packet rate saturates ~26 GB/s at
  15 KB -- 18 KB packets gain nothing), so end-time ~= last-data +
  drain + ~8 us exit barrier regardless of start phase. Small leading
  chunks DO start DVE ~5 us earlier but the gain returns as mid-run
  data-waits. Measured regressions -- do not retry: ACT Square+accum
  hybrid rows (shared-rel_t AND separate-out2 variants, ~+8 us); CCE
  DMA-subtract (runtime INTERNAL error); 30-row chunks with io bufs=4
  (late DMA starvation). Slow runs (~135-141 us) are a degraded
  fabric mode (~340 GB/s vs ~425 on the same NEFF) -- environmental;
  A/B-compare configs by the min of 2+ samples.
"""

import numpy as np

B, C, D = 256, 1000, 300
M = 8              # cores
BL = B // M        # 32 local anchors per core
ROWS = BL * C      # 32000 rows per core
P = 128            # partitions; partition p holds rows [p*RPP, (p+1)*RPP)
RPP = ROWS // P    # 250 rows per partition
RPC = 25           # max rows per chunk (per partition)
NCH = RPP // RPC   # 10 chunks
EPS_PD = 1e-6
MARGIN = 1.0

_STATE = {}


def _build():
    import concourse.tile as tile
    from concourse import bacc, mybir

    nc = bacc.Bacc("TRN2", target_bir_lowering=False, debug=False,
                   num_devices=M, num_swdge_queues=4)
    dt = mybir.dt.float32
    dt16 = mybir.dt.float16
    emb = nc.dram_tensor("emb", [ROWS, D], dt16, kind="ExternalInput").ap()
    att = nc.dram_tensor("att", [ROWS, D], dt16, kind="ExternalInput").ap()
    out = nc.dram_tensor("out", [P, RPP], dt, kind="ExternalOutput").ap()

    # row = p*250 + w  ->  per-partition row view [p, w, d]; the last two
    # chunks are small to shorten the serial drain chain after the final
    # DMA lands (sub -> Square -> folds on the last chunk is the tail)
    emb_v = emb.rearrange("(p w) d -> p w d", p=P, w=RPP)
    att_v = att.rearrange("(p w) d -> p w d", p=P, w=RPP)
    sizes = [10, 10] + [25] * 8 + [15, 10, 5]
    offs = np.cumsum([0] + sizes).tolist()
    assert offs[-1] == RPP

    Alu = mybir.AluOpType
    Act = mybir.ActivationFunctionType
    Ax = mybir.AxisListType

    with tile.TileContext(nc) as tc:
        with (
            tc.tile_pool(name="io", bufs=5) as io_pool,
            tc.tile_pool(name="tmp", bufs=5) as tmp_pool,
            tc.tile_pool(name="small", bufs=1) as small_pool,
        ):
            eps_t = small_pool.tile([P, 1], dt, tag="eps")
            nc.vector.memset(eps_t[:], EPS_PD)
            rel_t = small_pool.tile([P, RPP], dt, tag="rel")

            def folds(s_t, o, n, on_gpsimd=False):
                # f16 adds run 2 elem/cycle on DVE; the f32-accum reduce only
                # 1/cycle, so fold D 300->150->75 in f16 first. The tail
                # chunks' folds go to gpsimd (idle after descgen) so they
                # run in parallel with DVE draining its backlog after the
                # last DMA lands; DVE keeps only the tiny reduces.
                eng = nc.gpsimd if on_gpsimd else nc.vector
                f1_t = tmp_pool.tile([P, RPC, 150], dt16, tag="f1")
                eng.tensor_tensor(
                    f1_t[:, 0:n], s_t[:, 0:n, 0:150], s_t[:, 0:n, 150:300],
                    op=Alu.add)
                f2_t = tmp_pool.tile([P, RPC, 75], dt16, tag="f2")
                eng.tensor_tensor(
                    f2_t[:, 0:n], f1_t[:, 0:n, 0:75], f1_t[:, 0:n, 75:150],
                    op=Alu.add)
                nc.vector.tensor_reduce(
                    rel_t[:, o:o + n], f2_t[:, 0:n], axis=Ax.X, op=Alu.add)

            # software pipeline: chunk j's folds are emitted AFTER chunk
            # j+1's subtract. Engine streams execute in order, so without
            # the skew DVE sits idle inside every chunk waiting for ACT's
            # Square (f1 reads it) -- that serial chain gated DMA issue at
            # ~15 us/chunk.
            pend = None
            for j, n in enumerate(sizes):
                o = offs[j]
                e_t = io_pool.tile([P, RPC, D], dt16, tag="e")
                nc.gpsimd.dma_start(e_t[:, 0:n], emb_v[:, o:o + n])
                a_t = io_pool.tile([P, RPC, D], dt16, tag="a")
                nc.gpsimd.dma_start(a_t[:, 0:n], att_v[:, o:o + n])
                # in-place diff then Square back over e: no extra tiles, so
                # the io pool runs deep and DMA issue never waits on compute
                nc.vector.tensor_sub(a_t[:, 0:n], e_t[:, 0:n], a_t[:, 0:n])
                nc.scalar.activation(e_t[:, 0:n], a_t[:, 0:n], Act.Square,
                                     bias=eps_t[:], scale=1.0)
                if pend is not None:
                    folds(*pend)
                pend = (e_t, o, n) if j < len(sizes) - 3 else (e_t, o, n, True)
            folds(*pend)

            nc.sync.dma_start(out[:], rel_t[:])
    nc.compile()
    return nc


def _get_nc():
    if "nc" not in _STATE:
        _STATE["nc"] = _build()
    return _STATE["nc"]


def _run_device(attributes, embeddings, labels_np, trace=False):
    from concourse.bass_utils import run_bass_kernel_spmd
    nc = _get_nc()
    attributes = np.ascontiguousarray(attributes.astype(np.float16, copy=False))
    embeddings = np.ascontiguousarray(embeddings.astype(np.float16, copy=False))
    in_maps = []
    for m in range(M):
        sl = slice(m * ROWS, (m + 1) * ROWS)
        in_maps.append({
            "emb": embeddings[sl],
            "att": attributes[sl],
        })
    return run_bass_kernel_spmd(nc, in_maps, list(range(M)), trace=trace)


def _combine(results, labels_np):
    """Assemble [B, C] relations from per-core row-sums; finish on host."""
    rel_sq = np.concatenate(
        [np.asarray(r["out"], dtype=np.float64).reshape(ROWS) for r in results]
    ).reshape(B, C)
    relations = np.sqrt(np.maximum(rel_sq, 0.0))
    mask_pos = np.zeros((B, C), dtype=np.float64)
    mask_pos[np.arange(B), labels_np.astype(np.int64)] = 1.0
    hp = (relations * mask_pos).max(axis=0)
    mx = relations.max(axis=0)
    hn = (relations + mx[None, :] * mask_pos).min(axis=0)
    triplet = np.maximum(hp - hn + MARGIN, 0.0)
    num_hard = np.sum(triplet > 1e-16)
    loss = np.sum(triplet) / (num_hard + 1e-16)
    return np.float32(loss)


def kernel(attributes, embeddings, labels):
    attributes = np.asarray(attributes)
    embeddings = np.asarray(embeddings)
    labels_np = np.asarray(labels)
    res = _run_device(attributes, embeddings, labels_np)
    return _combine(res.results, labels_np)


# revision 52
# speedup vs baseline: 1.0387x; 1.0387x over previous
"""HardTripletLoss Trainium2 kernel.

Reference computation (B=256, C=1000, D=300):
  relations[b,c] = ||emb[b*C+c] - att[b*C+c] + 1e-6||_2          [B, C]
  hardest_positive[c] = max_b relations[b,c] * onehot(labels)[b,c]
  mx[c]              = max_b relations[b,c]
  hardest_negative[c] = min_b (relations[b,c] + mx[c]*onehot[b,c])
  loss = sum(relu(hp - hn + 1)) / (count(relu(...) > 1e-16) + 1e-16)

Sharding: data-parallel over B across 8 cores; core m owns the contiguous
32000-row slice [m*32000, (m+1)*32000) of the (B*C, D) tensors. The device
does the heavy part only: per-row squared distances rel_sq[row] =
sum_d (emb-att+eps)^2, shipped back as [128, 250] f32 (128 KB/core,
+0.3% traffic). The host reshapes to [B, C], takes sqrt, and runs the
reference's tiny [B,C] max/min/mask logic in numpy (~ms).

Performance design (HW exec ~128 us vs 610 us baseline):
- fp16 inputs (host pre-cast): halves device bytes to 38.4 MB/core.
  Final loss err ~3e-5 vs 2e-2 tolerance.
- All bulk loads on SWDGE (gpsimd): packets round-robin over all 16 DMA
  engines. HWDGE queues are pinned to engines 64-68 only and bottleneck
  at ~110 GB/s total.
- Rows-contiguous layout: partition p holds rows [p*250, (p+1)*250), so
  a 25-row chunk DMA is [128, 15 KB] -- 128 descriptors, 15 KB packets.
  Packet size sets the per-engine rate: 4.8 KB packets sustain ~200
  GB/s/core, 15 KB packets ~425 GB/s/core. Few descriptors per DMA also
  matters: SWDGE issue lock-steps on an 8-deep completion-sem ring, and
  250-desc DMAs stalled the issue stream 10-14 us at a time.
- Per chunk: in-place DVE subtract, one ACT Square (bias=eps), two f16
  fold-adds (2 elem/cycle) + one f32 3D row-sum reduce on DVE, emitted
  one chunk behind the subtract (engine streams are in-order; without
  the skew DVE idles waiting for ACT inside every chunk).
- ~127.6 us is the measured structural floor: DVE busy (~96 us; the
  reduce has no DVE fast modes, custom-DVE accum is [P,1]-only) ~=
  DMA stream (~92.5 us; per-engine packet rate saturates ~26 GB/s at
  15 KB -- 18 KB packets gain nothing), so end-time ~= last-data +
  drain + ~8 us exit barrier regardless of start phase. Small leading
  chunks DO start DVE ~5 us earlier but the gain returns as mid-run
  data-waits. Measured regressions -- do not retry: ACT Square+accum
  hybrid rows (shared-rel_t AND separate-out2 variants, ~+8 us); CCE
  DMA-subtract (runtime INTERNAL error); 30-row chunks with io bufs=4
  (late DMA starvation). Slow runs (~135-141 us) are a degraded
  fabric mode (~340 GB/s vs ~425 on the same NEFF) -- environmental;
  A/B-compare configs by the min of 2+ samples.
"""

import numpy as np

B, C, D = 256, 1000, 300
M = 8              # cores
BL = B // M        # 32 local anchors per core
ROWS = BL * C      # 32000 rows per core
P = 128            # partitions; partition p holds rows [p*RPP, (p+1)*RPP)
RPP = ROWS // P    # 250 rows per partition
RPC = 25           # max rows per chunk (per partition)
NCH = RPP // RPC   # 10 chunks
EPS_PD = 1e-6
MARGIN = 1.0

_STATE = {}


def _build():
    import concourse.tile as tile
    from concourse import bacc, mybir

    nc = bacc.Bacc("TRN2", target_bir_lowering=False, debug=False,
                   num_devices=M, num_swdge_queues=4)
    dt = mybir.dt.float32
    dt16 = mybir.dt.float16
    emb = nc.dram_tensor("emb", [ROWS, D], dt16, kind="ExternalInput").ap()
    att = nc.dram_tensor("att", [ROWS, D], dt16, kind="ExternalInput").ap()
    out = nc.dram_tensor("out", [P, RPP], dt, kind="ExternalOutput").ap()

    # row = p*250 + w  ->  per-partition row view [p, w, d]; the last two
    # chunks are small to shorten the serial drain chain after the final
    # DMA lands (sub -> Square -> folds on the last chunk is the tail)
    emb_v = emb.rearrange("(p w) d -> p w d", p=P, w=RPP)
    att_v = att.rearrange("(p w) d -> p w d", p=P, w=RPP)
    sizes = [10, 10] + [25] * 8 + [15, 10, 5]
    offs = np.cumsum([0] + sizes).tolist()
    assert offs[-1] == RPP

    Alu = mybir.AluOpType
    Act = mybir.ActivationFunctionType
    Ax = mybir.AxisListType

    with tile.TileContext(nc) as tc:
        with (
            tc.tile_pool(name="io", bufs=5) as io_pool,
            tc.tile_pool(name="tmp", bufs=5) as tmp_pool,
            tc.tile_pool(name="small", bufs=1) as small_pool,
        ):
            eps_t = small_pool.tile([P, 1], dt, tag="eps")
            nc.vector.memset(eps_t[:], EPS_PD)
            rel_t = small_pool.tile([P, RPP], dt, tag="rel")

            def folds(s_t, o, n):
                # f16 adds run 2 elem/cycle on DVE; the f32-accum reduce only
                # 1/cycle, so fold D 300->150->75 in f16 first
                f1_t = tmp_pool.tile([P, RPC, 150], dt16, tag="f1")
                nc.vector.tensor_tensor(
                    f1_t[:, 0:n], s_t[:, 0:n, 0:150], s_t[:, 0:n, 150:300],
                    op=Alu.add)
                f2_t = tmp_pool.tile([P, RPC, 75], dt16, tag="f2")
                nc.vector.tensor_tensor(
                    f2_t[:, 0:n], f1_t[:, 0:n, 0:75], f1_t[:, 0:n, 75:150],
                    op=Alu.add)
                nc.vector.tensor_reduce(
                    rel_t[:, o:o + n], f2_t[:, 0:n], axis=Ax.X, op=Alu.add)

            # software pipeline: chunk j's folds are emitted AFTER chunk
            # j+1's subtract. Engine streams execute in order, so without
            # the skew DVE sits idle inside every chunk waiting for ACT's
            # Square (f1 reads it) -- that serial chain gated DMA issue at
            # ~15 us/chunk.
            pend = None
            for j, n in enumerate(sizes):
                o = offs[j]
                e_t = io_pool.tile([P, RPC, D], dt16, tag="e")
                nc.gpsimd.dma_start(e_t[:, 0:n], emb_v[:, o:o + n])
                a_t = io_pool.tile([P, RPC, D], dt16, tag="a")
                nc.gpsimd.dma_start(a_t[:, 0:n], att_v[:, o:o + n])
                # in-place diff then Square back over e: no extra tiles, so
                # the io pool runs deep and DMA issue never waits on compute
                nc.vector.tensor_sub(a_t[:, 0:n], e_t[:, 0:n], a_t[:, 0:n])
                nc.scalar.activation(e_t[:, 0:n], a_t[:, 0:n], Act.Square,
                                     bias=eps_t[:], scale=1.0)
                if pend is not None:
                    folds(*pend)
                pend = (e_t, o, n)
            folds(*pend)

            nc.sync.dma_start(out[:], rel_t[:])
    nc.compile()
    return nc


def _get_nc():
    if "nc" not in _STATE:
        _STATE["nc"] = _build()
    return _STATE["nc"]


def _run_device(attributes, embeddings, labels_np, trace=False):
    from concourse.bass_utils import run_bass_kernel_spmd
    nc = _get_nc()
    attributes = np.ascontiguousarray(attributes.astype(np.float16, copy=False))
    embeddings = np.ascontiguousarray(embeddings.astype(np.float16, copy=False))
    in_maps = []
    for m in range(M):
        sl = slice(m * ROWS, (m + 1) * ROWS)
        in_maps.append({
            "emb": embeddings[sl],
            "att": attributes[sl],
        })
    return run_bass_kernel_spmd(nc, in_maps, list(range(M)), trace=trace)


def _combine(results, labels_np):
    """Assemble [B, C] relations from per-core row-sums; finish on host."""
    rel_sq = np.concatenate(
        [np.asarray(r["out"], dtype=np.float64).reshape(ROWS) for r in results]
    ).reshape(B, C)
    relations = np.sqrt(np.maximum(rel_sq, 0.0))
    mask_pos = np.zeros((B, C), dtype=np.float64)
    mask_pos[np.arange(B), labels_np.astype(np.int64)] = 1.0
    hp = (relations * mask_pos).max(axis=0)
    mx = relations.max(axis=0)
    hn = (relations + mx[None, :] * mask_pos).min(axis=0)
    triplet = np.maximum(hp - hn + MARGIN, 0.0)
    num_hard = np.sum(triplet > 1e-16)
    loss = np.sum(triplet) / (num_hard + 1e-16)
    return np.float32(loss)


def kernel(attributes, embeddings, labels):
    attributes = np.asarray(attributes)
    embeddings = np.asarray(embeddings)
    labels_np = np.asarray(labels)
    res = _run_device(attributes, embeddings, labels_np)
    return _combine(res.results, labels_np)
